# revision 3
# baseline (speedup 1.0000x reference)
"""Multi-head attention forward on 8 Trainium2 NeuronCores (Bass/Tile).

Problem: B=4, N=2048, C=1024, H=16, D=64.
    qkv = x @ w_qkv ; per-head scaled softmax(q k^T) v ; o @ w_proj + b_proj

Sharding: core c handles batch (c // 2) and heads (c % 2)*8 .. +8.
ONE SPMD launch per core (the old separate o@w_proj launch is merged in):

  - per-core qkv projection + flash-style attention over its 8 heads
    (4 head-pairs) of its batch.  All matmul operands are bf16.  Structure
    per head-pair hp:
      - project k, v, q for hp (PE; interleaved in program order with
        the attention of head-pair hp-1),
      - attention: per (qb, kv): S^T = k^T q (2 matmuls, K=64, M=128,
        F=512 into one [128,2,512] PSUM tile), exp on ACT ([128,1024]
        per instruction, PSUM -> SBUF bf16), then PV with the exp
        output as lhsT: out[q=128, 65] += P V_aug (K=128, M=128, F=65).
        V is augmented with a ones column so row sums (softmax
        denominators) fall out of the same matmuls.
  - Epilogue per (qb, hp): DVE reciprocal of the denominator column,
    normalize-on-copy (PSUM fp32 -> SBUF bf16, tensor_scalar multiply
    by 1/den per q-subtile), then 4 PE transposes (identity matmul,
    one shared-pool PSUM bank) build oT[qb][:, hp, :] = o_norm^T.
  - After head-pair 3 finishes q-block qb: 16 output-projection
    tenures out_partial[qsl, :] = oT_qb^T(512 local feats) @ w_proj
    rotate through the same 2-bank PSUM proj pool.  DMA out fp32.
  (host) out[b] = partial[2b] + partial[2b+1] + b_proj  (free).

  PSUM budget: st 2x2 banks + acc 2x1 + proj/transpose/oproj pool 2x1
  = 8 banks exactly.

Cost model per core: PE = proj 196608 + scores 262144 + PV 133120
+ transposes 8192 + oproj 65536 = 665600 cycles = 277.3us @2.4GHz (the
bound); ACT exp 256 x 1.04us = 265.7us just fits under it.  The merge
removes the old launch-2 (40.1us) entirely.
"""

import numpy as np

import concourse.bacc as bacc
import concourse.bass as bass
import concourse.tile as tile
from concourse import mybir

F32 = mybir.dt.float32
BF16 = mybir.dt.bfloat16
NP_BF16 = mybir.dt.np(mybir.dt.bfloat16)

B, N, C, H = 4, 2048, 1024, 16
D = C // H          # 64
NCORES = 8
HL = H // 2         # 8 local heads per core
FL = HL * D         # 512 local features
KO = C // 128       # 8 contraction tiles (qkv proj)
KO2 = FL // 128     # 4 contraction tiles (out proj)
TB = N // 512       # 4 token blocks of 512
KV = N // 128       # 16 kv tiles of 128
QB = N // 512       # 4 query blocks of 512


# tuning knobs (timeline-sim sweeps)
PT_BUFS = 10
ST_BUFS = 2
PROJ_BUFS = 2
PV_PRIO_OFFSET = 60


# ---------------------------------------------------------------- launch
def _build_l1(reps=1):
    nc = bacc.Bacc("TRN2", target_bir_lowering=False, debug=False)
    xt = nc.dram_tensor("xt", [C, N], BF16, kind="ExternalInput")
    wq = nc.dram_tensor("wq", [C, FL], BF16, kind="ExternalInput")
    wk = nc.dram_tensor("wk", [C, FL], BF16, kind="ExternalInput")
    wv = nc.dram_tensor("wv", [C, FL], BF16, kind="ExternalInput")
    wp = nc.dram_tensor("wp", [FL, C], BF16, kind="ExternalInput")
    ident = nc.dram_tensor("ident", [128, 128], BF16, kind="ExternalInput")
    out = nc.dram_tensor("out", [N, C], F32, kind="ExternalOutput")

    xt_r = xt.ap().rearrange("(o p) n -> p o n", p=128)
    wq_r = wq.ap().rearrange("(o p) f -> p o f", p=128)
    wk_r = wk.ap().rearrange("(o p) f -> p o f", p=128)
    wv_r = wv.ap().rearrange("(o p) f -> p o f", p=128)
    wp_r = wp.ap().rearrange("(o p) c -> p o c", p=128)

    with (
        tile.TileContext(nc) as tc,
        tc.tile_pool(name="persist", bufs=1) as persist,
        tc.tile_pool(name="qk", bufs=2) as qk_p,
        tc.tile_pool(name="vp", bufs=2) as v_p,
        tc.tile_pool(name="pt", bufs=PT_BUFS) as pt_p,
        tc.tile_pool(name="onorm", bufs=4) as onorm_p,
        tc.tile_pool(name="rden", bufs=2) as rden_p,
        tc.tile_pool(name="outp", bufs=4) as outp,
        tc.tile_pool(name="ps_proj", bufs=PROJ_BUFS, space="PSUM") as ps_proj,
        tc.tile_pool(name="ps_st", bufs=ST_BUFS, space="PSUM") as ps_st,
        tc.tile_pool(name="ps_acc", bufs=1, space="PSUM") as ps_acc,
    ):
        xt_sb = persist.tile([128, KO, N], BF16)
        wq_sb = persist.tile([128, KO, FL], BF16)
        wk_sb = persist.tile([128, KO, FL], BF16)
        wv_sb = persist.tile([128, KO, FL], BF16)
        wp_sb = persist.tile([128, KO2, C], BF16)
        id_sb = persist.tile([128, 128], BF16)
        # oT[qb]: o_norm^T [512 local feats (4 ko-chunks of 128), 512 toks]
        oT = [persist.tile([128, KO2, 512], BF16, name=f"oT{qb}")
              for qb in range(QB)]
        # All loads on the SP queue in just-in-time order for the first
        # attention sweep (transfers serialize on the shared DMA engines, so
        # the queue order IS the arrival order): the head-pair-0 column
        # slices of the weights (cheap 128-col loads) + xt(tb0) enable the
        # first k/q/v chunks by ~6us, xt(tb1..3) arrive right before the
        # k(tb1..3) chunks need them, and the remaining weight columns
        # trail in (first needed by head-pair 1, ~70us later).  w_proj and
        # the transpose identity are first needed at the hp0-qb0 epilogue
        # (~80us in), so they ride at the back.
        nc.sync.dma_start(wk_sb[:, :, 0:128], wk_r[:, :, 0:128])
        nc.sync.dma_start(xt_sb[:, :, 0:256], xt_r[:, :, 0:256])
        nc.sync.dma_start(xt_sb[:, :, 256:512], xt_r[:, :, 256:512])
        nc.sync.dma_start(wq_sb[:, :, 0:128], wq_r[:, :, 0:128])
        nc.sync.dma_start(wv_sb[:, :, 0:128], wv_r[:, :, 0:128])
        for hb in range(2, 2 * TB):
            nc.sync.dma_start(xt_sb[:, :, hb * 256:(hb + 1) * 256],
                              xt_r[:, :, hb * 256:(hb + 1) * 256])
        nc.sync.dma_start(wk_sb[:, :, 128:], wk_r[:, :, 128:])
        nc.sync.dma_start(wq_sb[:, :, 128:], wq_r[:, :, 128:])
        nc.sync.dma_start(wv_sb[:, :, 128:], wv_r[:, :, 128:])
        nc.sync.dma_start(id_sb[:], ident.ap())
        nc.sync.dma_start(wp_sb[:], wp_r[:])

        for _rep in range(reps):
            def proj_chunks(hp, qT, kT, vA):
                """Generator: project k, v, q of head-pair hp in ~1us chunks.

                Chunk order is just-in-time for the first attention sweep
                (qb0 over kv 0..15): k(tb0) and q(tb0) first so scores can
                start immediately, then v in kv order interleaved with the
                remaining k blocks, then the remaining q blocks.
                """
                fsl = slice(hp * 128, (hp + 1) * 128)

                def kq(w_sb, dstT, tb, nm, half):
                    tok = slice(tb * 512 + half * 256,
                                tb * 512 + (half + 1) * 256)
                    p = ps_proj.tile([128, 256], F32, tag="proj",
                                     name=f"ps{nm}{hp}_{tb}_{half}")
                    for ko in range(KO):
                        nc.tensor.matmul(p[:], w_sb[:, ko, fsl],
                                         xt_sb[:, ko, tok],
                                         start=(ko == 0), stop=(ko == KO - 1))
                    nc.vector.tensor_copy(dstT[:, tok], p[:])

                def v_tile(tt):
                    tok = slice(tt * 128, (tt + 1) * 128)
                    psv = ps_proj.tile([128, 128], F32, tag="proj",
                                       name=f"psv{hp}_{tt}")
                    for ko in range(KO):
                        nc.tensor.matmul(psv[:], xt_sb[:, ko, tok],
                                         wv_sb[:, ko, fsl],
                                         start=(ko == 0), stop=(ko == KO - 1))
                    dst = vA[:, tt, :].rearrange("p (l c) -> p l c", l=2)
                    src = psv.rearrange("p (l c) -> p l c", l=2)
                    nc.vector.tensor_copy(dst[:, :, 0:64], src[:])

                for half in (0, 1):
                    kq(wk_sb, kT, 0, "k", half)
                    yield
                for half in (0, 1):
                    kq(wq_sb, qT, 0, "q", half)
                    yield
                nc.vector.memset(vA[:, :, 64], 1.0)
                nc.vector.memset(vA[:, :, 129], 1.0)
                for grp in range(TB):
                    if grp > 0:
                        for half in (0, 1):
                            kq(wk_sb, kT, grp, "k", half)
                            yield
                    for tt in range(grp * 4, grp * 4 + 4):
                        v_tile(tt)
                        yield
                for tb in range(1, TB):
                    for half in (0, 1):
                        kq(wq_sb, qT, tb, "q", half)
                        yield

            def attn_steps(hp, qT, kT, vA):
                """Generator: attention for head-pair hp, one kv step or one
                epilogue per yield.

                The PV accumulators hold 4 q-subtile chains per PSUM bank.
                A matmul with start=True lazily zeroes its bank's ENTIRE 2KB
                zero region, so interleaved chains in one bank would wipe
                each other's partials (and the scheduler is free to
                interleave disjoint-region writers).  Instead the tiles are
                zeroed once with a DVE memset and every PV matmul
                accumulates (start=False, group check off) -- adds commute,
                so any execution order is correct.
                """
                for qb in range(QB):
                    qsl = slice(qb * 512, (qb + 1) * 512)
                    accs = [ps_acc.tile([128, 4, 65], F32, tag=f"acc{h}",
                                        name=f"acc{h}_{hp}_{qb}")
                            for h in (0, 1)]
                    for h in (0, 1):
                        nc.vector.memset(accs[h][:], 0.0)
                    for kv in range(KV):
                        ksl = slice(kv * 128, (kv + 1) * 128)
                        st = ps_st.tile([128, 2, 512], F32, tag="st",
                                        name=f"st{hp}_{qb}_{kv}")
                        for h in (0, 1):
                            hsl = slice(h * 64, (h + 1) * 64)
                            nc.tensor.matmul(st[:, h, :], kT[hsl, ksl],
                                             qT[hsl, qsl],
                                             start=True, stop=True)
                        pt = pt_p.tile([128, 2, 512], BF16, tag="pt",
                                       name=f"pt{hp}_{qb}_{kv}")
                        nc.scalar.activation(pt[:], st[:],
                                             mybir.ActivationFunctionType.Exp)
                        # Deprioritize PV: when both are ready the PE should
                        # run the ACT-feeding score/proj work first; the pt
                        # pool gives PV ~PT_BUFS steps of laxity and pt-slot
                        # pressure self-balances.
                        po = tc.cur_priority
                        tc.cur_priority = po + PV_PRIO_OFFSET
                        for h in (0, 1):
                            vsl = slice(h * 65, (h + 1) * 65)
                            for sub in range(4):
                                nc.tensor.matmul(
                                    accs[h][:, sub, :],
                                    pt[:, h, sub * 128:(sub + 1) * 128],
                                    vA[:, kv, vsl],
                                    start=False, stop=(kv == KV - 1),
                                    skip_group_check=True)
                        tc.cur_priority = po
                        yield
                    # Epilogue: normalize o = num/den on the PSUM->SBUF
                    # copy, then transpose to oT[qb][:, hp, :].
                    rden = rden_p.tile([128, 2, 4], F32, tag="rden",
                                       name=f"rden{hp}_{qb}")
                    onrm = onorm_p.tile([128, 4, 2, 64], BF16, tag="onorm",
                                        name=f"onrm{hp}_{qb}")
                    nc.vector.reciprocal(rden[:, 0, :], accs[0][:, :, 64])
                    for sub in range(4):
                        nc.vector.tensor_scalar_mul(
                            onrm[:, sub, 0, :], accs[0][:, sub, 0:64],
                            rden[:, 0, sub:sub + 1])
                    yield
                    nc.vector.reciprocal(rden[:, 1, :], accs[1][:, :, 64])
                    for sub in range(4):
                        nc.vector.tensor_scalar_mul(
                            onrm[:, sub, 1, :], accs[1][:, sub, 0:64],
                            rden[:, 1, sub:sub + 1])
                    tp = ps_proj.tile([128, 4, 128], BF16, tag="proj",
                                      name=f"tp{hp}_{qb}")
                    for sub in range(4):
                        nc.tensor.matmul(tp[:, sub, :],
                                         onrm[:, sub, :, :], id_sb[:],
                                         is_transpose=True,
                                         start=(sub == 0), stop=(sub == 3),
                                         skip_group_check=True)
                    nc.vector.tensor_copy(
                        oT[qb][:, hp, :],
                        tp.rearrange("p a b -> p (a b)"))
                    yield

            def oproj_steps(qb):
                """Generator: partial out[qsl, :] = oT_qb^T @ w_proj_local,
                one [128 tok, 256 cout] PSUM tenure per yield."""
                for tt in range(4):
                    tsl = slice(tt * 128, (tt + 1) * 128)
                    osl = slice(qb * 512 + tt * 128, qb * 512 + (tt + 1) * 128)
                    for co in range(4):
                        csl = slice(co * 256, (co + 1) * 256)
                        ps = ps_proj.tile([128, 256], F32, tag="proj",
                                          name=f"op{qb}_{tt}_{co}")
                        for ko in range(KO2):
                            nc.tensor.matmul(ps[:], oT[qb][:, ko, tsl],
                                             wp_sb[:, ko, csl],
                                             start=(ko == 0),
                                             stop=(ko == KO2 - 1))
                        ob = outp.tile([128, 256], F32, tag="o",
                                       name=f"ob{qb}_{tt}_{co}")
                        nc.vector.tensor_copy(ob[:], ps[:])
                        nc.sync.dma_start(out.ap()[osl, csl], ob[:])
                        yield

            def hp_tiles(hp):
                qT = qk_p.tile([128, N], BF16, tag="qT", name=f"qT{hp}")
                kT = qk_p.tile([128, N], BF16, tag="kT", name=f"kT{hp}")
                vA = v_p.tile([128, KV, 130], BF16, tag="vA", name=f"vA{hp}")
                return qT, kT, vA

            # Interleave projection chunks (~1 per attention step) with the
            # attention steps; the tile scheduler resolves real deps, the
            # emission order sets priorities.  Attention(hp0) is emitted
            # right after k(tb0)+q(tb0) so the ACT pipeline starts ~10us in.
            # Emission order IS dependency order for the tile framework: an
            # attention step must be emitted AFTER the proj chunks it reads.
            # need_chunks[step] = how many chunks of the CURRENT head-pair's
            # generator must be emitted before attention step `step` (chunk
            # order: k0 q0 v0-3 k1 v4-7 k2 v8-11 k3 v12-15 q1 q2 q3).
            vpos = [4, 5, 6, 7, 10, 11, 12, 13, 16, 17, 18, 19, 22, 23, 24, 25]

            def need_chunks(step):
                qb, within = divmod(step, KV + 2)
                if qb == 0:
                    return vpos[min(within, KV - 1)] + 1
                return 26 + 2 * min(qb, 3)

            from collections import deque
            cur = hp_tiles(0)
            pending = deque([[0, proj_chunks(0, *cur), 0]])

            def pull_one():
                while pending:
                    ent = pending[0]
                    if next(ent[1], StopIteration) is StopIteration:
                        pending.popleft()
                    else:
                        ent[2] += 1
                        return
            for hp in range(NHP := HL // 2):
                if hp < NHP - 1:
                    nxt = hp_tiles(hp + 1)
                    pending.append([hp + 1, proj_chunks(hp + 1, *nxt), 0])
                else:
                    nxt = None
                agen = attn_steps(hp, *cur)
                for step in range(QB * (KV + 2)):
                    # hard requirement: current head-pair's chunks this
                    # attention step reads must already be emitted
                    while (pending and pending[0][0] == hp
                           and pending[0][2] < need_chunks(step)):
                        pull_one()
                    # cadence fill: one chunk every 3rd step regardless of
                    # owner -- the need-driven pulls above already force
                    # everything an attention step reads, so eager draining
                    # only crowds the PE and slips ACT.  During the last
                    # head-pair the queue holds oproj tenures and there are
                    # no more ACT-feeding proj chunks, so drain every step.
                    if pending and (step % 3 == 0 or hp == NHP - 1):
                        pull_one()
                    next(agen, None)
                    if hp == NHP - 1 and step % (KV + 2) == KV + 1:
                        qb_done = step // (KV + 2)
                        pending.append([-1, oproj_steps(qb_done), 0])
                cur = nxt
            while pending:
                if next(pending[0][1], StopIteration) is StopIteration:
                    pending.popleft()

    nc.compile()
    return nc


# ---------------------------------------------------------------- runner
class _SpmdRunner:
    """jit-once SPMD runner over n cores (modeled on bass2jax.run_bass_via_pjrt)."""

    def __init__(self, nc, n_cores):
        import jax
        from jax.experimental.shard_map import shard_map
        from jax.sharding import Mesh, PartitionSpec
        from concourse.bass2jax import (_bass_exec_p, install_neuronx_cc_hook,
                                        partition_id_tensor)

        install_neuronx_cc_hook()
        self.jax = jax
        self.n_cores = n_cores
        partition_name = (nc.partition_id_tensor.name
                          if nc.partition_id_tensor else None)
        in_names, out_names, out_avals, zero_shapes = [], [], [], []
        for alloc in nc.m.functions[0].allocations:
            if not isinstance(alloc, mybir.MemoryLocationSet):
                continue
            name = alloc.memorylocations[0].name
            if alloc.kind == "ExternalInput":
                if name != partition_name:
                    in_names.append(name)
            elif alloc.kind == "ExternalOutput":
                shape = tuple(alloc.tensor_shape)
                dtype = mybir.dt.np(alloc.dtype)
                out_names.append(name)
                out_avals.append(jax.core.ShapedArray(shape, dtype))
                zero_shapes.append((shape, dtype))
        self.in_names, self.out_names = in_names, out_names
        self.out_avals, self.zero_shapes = out_avals, zero_shapes
        n_params, n_outs = len(in_names), len(out_names)
        all_in = list(in_names) + list(out_names)
        if partition_name is not None:
            all_in.append(partition_name)

        def _body(*args):
            operands = list(args)
            if partition_name is not None:
                operands.append(partition_id_tensor())
            return tuple(_bass_exec_p.bind(
                *operands, out_avals=tuple(out_avals), in_names=tuple(all_in),
                out_names=tuple(out_names), lowering_input_output_aliases=(),
                sim_require_finite=True, sim_require_nnan=True, nc=nc))

        devices = jax.devices()[:n_cores]
        self.mesh = Mesh(np.asarray(devices), ("core",))
        self.pspec = PartitionSpec("core")
        in_specs = (self.pspec,) * (n_params + n_outs)
        out_specs = (self.pspec,) * n_outs
        self.fn = jax.jit(
            shard_map(_body, mesh=self.mesh, in_specs=in_specs,
                      out_specs=out_specs, check_rep=False),
            donate_argnums=tuple(range(n_params, n_params + n_outs)),
            keep_unused=True)

    def _stage(self, in_maps):
        from jax.sharding import NamedSharding
        sharding = NamedSharding(self.mesh, self.pspec)
        concat = [np.concatenate([np.asarray(m[n]) for m in in_maps], axis=0)
                  for n in self.in_names]
        dev_in = [self.jax.device_put(x, sharding) for x in concat]
        for x in dev_in:
            x.block_until_ready()
        return sharding, dev_in

    def _zeros(self, sharding):
        zeros = [self.jax.device_put(
            np.zeros((self.n_cores * s[0], *s[1:]), d), sharding)
            for (s, d) in self.zero_shapes]
        for z in zeros:
            z.block_until_ready()
        return zeros

    def _unpack(self, outs):
        np_outs = [np.asarray(o) for o in outs]
        return [
            {n: np_outs[i].reshape(self.n_cores, *self.out_avals[i].shape)[c]
             for i, n in enumerate(self.out_names)}
            for c in range(self.n_cores)
        ]

    def run(self, in_maps):
        sharding, dev_in = self._stage(in_maps)
        outs = self.fn(*dev_in, *self._zeros(sharding))
        return self._unpack(outs)

    def timed_run(self, in_maps, iters=6):
        """Stage inputs once; time only execute+sync per iteration."""
        import time
        sharding, dev_in = self._stage(in_maps)
        walls = []
        outs = None
        for _ in range(iters):
            zeros = self._zeros(sharding)
            t0 = time.perf_counter()
            outs = self.fn(*dev_in, *zeros)
            for o in outs:
                o.block_until_ready()
            walls.append(time.perf_counter() - t0)
        return self._unpack(outs), walls


_STATE = {}


def _get_state():
    if "l1" not in _STATE:
        nc1 = _build_l1()
        _STATE["l1"] = nc1
        _STATE["r1"] = _SpmdRunner(nc1, NCORES)
    return _STATE


def _l1_in_maps(x, w_qkv, w_proj):
    scale = np.float32(D ** -0.5)
    ident = np.eye(128, dtype=NP_BF16)
    in_maps = []
    for c in range(NCORES):
        b = c // 2
        hg = c % 2
        fsl = slice(hg * FL, (hg + 1) * FL)
        in_maps.append({
            "xt": np.ascontiguousarray(x[b].T).astype(NP_BF16),
            "wq": (np.ascontiguousarray(w_qkv[:, fsl]) * scale).astype(NP_BF16),
            "wk": np.ascontiguousarray(w_qkv[:, C:][:, fsl]).astype(NP_BF16),
            "wv": np.ascontiguousarray(w_qkv[:, 2 * C:][:, fsl]).astype(NP_BF16),
            "wp": np.ascontiguousarray(w_proj[fsl, :]).astype(NP_BF16),
            "ident": ident,
        })
    return in_maps


def kernel(x, w_qkv, w_proj, b_proj):
    st = _get_state()
    x = np.asarray(x, dtype=np.float32)
    w_qkv = np.asarray(w_qkv, dtype=np.float32)
    w_proj = np.asarray(w_proj, dtype=np.float32)
    b_proj = np.asarray(b_proj, dtype=np.float32)

    res = st["r1"].run(_l1_in_maps(x, w_qkv, w_proj))

    # host: sum the two head-group partials per batch, add bias
    out = np.empty((B, N, C), dtype=np.float32)
    for b in range(B):
        out[b] = res[2 * b]["out"] + res[2 * b + 1]["out"] + b_proj
    return out


# revision 7
# speedup vs baseline: 1.0124x; 1.0124x over previous
"""Multi-head attention forward on 8 Trainium2 NeuronCores (Bass/Tile).

Problem: B=4, N=2048, C=1024, H=16, D=64.
    qkv = x @ w_qkv ; per-head scaled softmax(q k^T) v ; o @ w_proj + b_proj

Sharding: core c handles batch (c // 2) and heads (c % 2)*8 .. +8.
ONE SPMD launch per core (the old separate o@w_proj launch is merged in):

  - per-core qkv projection + flash-style attention over its 8 heads
    (4 head-pairs) of its batch.  All matmul operands are bf16.  Structure
    per head-pair hp:
      - project k, v, q for hp (PE; interleaved in program order with
        the attention of head-pair hp-1),
      - attention: per (qb, kv): S^T = k^T q (2 matmuls, K=64, M=128,
        F=512 into one [128,2,512] PSUM tile), exp on ACT ([128,1024]
        per instruction, PSUM -> SBUF bf16), then PV with the exp
        output as lhsT: out[q=128, 65] += P V_aug (K=128, M=128, F=65).
        V is augmented with a ones column so row sums (softmax
        denominators) fall out of the same matmuls.
  - Epilogue per (qb, hp): DVE reciprocal of the denominator column,
    normalize-on-copy (PSUM fp32 -> SBUF bf16, tensor_scalar multiply
    by 1/den per q-subtile), then 4 PE transposes (identity matmul,
    one shared-pool PSUM bank) build oT[qb][:, hp, :] = o_norm^T.
  - After head-pair 3 finishes q-block qb: 16 output-projection
    tenures out_partial[qsl, :] = oT_qb^T(512 local feats) @ w_proj
    rotate through the same 2-bank PSUM proj pool.  DMA out fp32.
  (host) out[b] = partial[2b] + partial[2b+1] + b_proj  (free).

  PSUM budget: st 2x2 banks + acc 2x1 + proj/transpose/oproj pool 2x1
  = 8 banks exactly.

Cost model per core: PE = proj 196608 + scores 262144 + PV 133120
+ transposes 8192 + oproj 65536 = 665600 cycles = 277.3us @2.4GHz (the
bound); ACT exp 256 x 1.04us = 265.7us just fits under it.  The merge
removes the old launch-2 (40.1us) entirely.
"""

import numpy as np

import concourse.bacc as bacc
import concourse.bass as bass
import concourse.tile as tile
from concourse import mybir

F32 = mybir.dt.float32
BF16 = mybir.dt.bfloat16
I16 = mybir.dt.int16
NP_BF16 = mybir.dt.np(mybir.dt.bfloat16)

# Schraudolph exp in bf16-bit domain: bf16_bits(exp(s)) ~= s*128/ln2 + 127*128
# + corr.  Computed as int16 on the DVE for 2 of the 16 kv steps per sweep,
# relieving the ACT exp bottleneck (GPSIMD cannot access PSUM, so the DVE
# takes both).  corr = -7 minimizes rms rel err (1.8%); +0.5 compensates the
# truncating float->int16 convert.  rel-err impact at 2/16 offload: ~+1e-3.
SCH_A = float(128.0 / np.log(2.0))
SCH_B = float(127 * 128 - 7 + 0.5)
SCH_KV = {5: "dve", 11: "dve"}

B, N, C, H = 4, 2048, 1024, 16
D = C // H          # 64
NCORES = 8
HL = H // 2         # 8 local heads per core
FL = HL * D         # 512 local features
KO = C // 128       # 8 contraction tiles (qkv proj)
KO2 = FL // 128     # 4 contraction tiles (out proj)
TB = N // 512       # 4 token blocks of 512
KV = N // 128       # 16 kv tiles of 128
QB = N // 512       # 4 query blocks of 512


# tuning knobs (timeline-sim sweeps)
PT_BUFS = 10
ST_BUFS = 2
PROJ_BUFS = 2
PV_PRIO_OFFSET = 60


# ---------------------------------------------------------------- launch
def _build_l1(reps=1):
    nc = bacc.Bacc("TRN2", target_bir_lowering=False, debug=False)
    xt = nc.dram_tensor("xt", [C, N], BF16, kind="ExternalInput")
    wq = nc.dram_tensor("wq", [C, FL], BF16, kind="ExternalInput")
    wk = nc.dram_tensor("wk", [C, FL], BF16, kind="ExternalInput")
    wv = nc.dram_tensor("wv", [C, FL], BF16, kind="ExternalInput")
    wp = nc.dram_tensor("wp", [FL, C], BF16, kind="ExternalInput")
    ident = nc.dram_tensor("ident", [128, 128], BF16, kind="ExternalInput")
    out = nc.dram_tensor("out", [N, C], F32, kind="ExternalOutput")

    xt_r = xt.ap().rearrange("(o p) n -> p o n", p=128)
    wq_r = wq.ap().rearrange("(o p) f -> p o f", p=128)
    wk_r = wk.ap().rearrange("(o p) f -> p o f", p=128)
    wv_r = wv.ap().rearrange("(o p) f -> p o f", p=128)
    wp_r = wp.ap().rearrange("(o p) c -> p o c", p=128)

    with (
        tile.TileContext(nc) as tc,
        tc.tile_pool(name="persist", bufs=1) as persist,
        tc.tile_pool(name="qk", bufs=2) as qk_p,
        tc.tile_pool(name="vp", bufs=2) as v_p,
        tc.tile_pool(name="pt", bufs=PT_BUFS) as pt_p,
        tc.tile_pool(name="onorm", bufs=4) as onorm_p,
        tc.tile_pool(name="rden", bufs=2) as rden_p,
        tc.tile_pool(name="outp", bufs=4) as outp,
        tc.tile_pool(name="ps_proj", bufs=PROJ_BUFS, space="PSUM") as ps_proj,
        tc.tile_pool(name="ps_st", bufs=ST_BUFS, space="PSUM") as ps_st,
        tc.tile_pool(name="ps_acc", bufs=1, space="PSUM") as ps_acc,
    ):
        xt_sb = persist.tile([128, KO, N], BF16)
        wq_sb = persist.tile([128, KO, FL], BF16)
        wk_sb = persist.tile([128, KO, FL], BF16)
        wv_sb = persist.tile([128, KO, FL], BF16)
        wp_sb = persist.tile([128, KO2, C], BF16)
        id_sb = persist.tile([128, 128], BF16)
        # oT[qb]: o_norm^T [512 local feats (4 ko-chunks of 128), 512 toks]
        oT = [persist.tile([128, KO2, 512], BF16, name=f"oT{qb}")
              for qb in range(QB)]
        # All loads on the SP queue in just-in-time order for the first
        # attention sweep (transfers serialize on the shared DMA engines, so
        # the queue order IS the arrival order): the head-pair-0 column
        # slices of the weights (cheap 128-col loads) + xt(tb0) enable the
        # first k/q/v chunks by ~6us, xt(tb1..3) arrive right before the
        # k(tb1..3) chunks need them, and the remaining weight columns
        # trail in (first needed by head-pair 1, ~70us later).  w_proj and
        # the transpose identity are first needed at the hp0-qb0 epilogue
        # (~80us in), so they ride at the back.
        nc.sync.dma_start(wk_sb[:, :, 0:128], wk_r[:, :, 0:128])
        nc.sync.dma_start(xt_sb[:, :, 0:256], xt_r[:, :, 0:256])
        nc.sync.dma_start(xt_sb[:, :, 256:512], xt_r[:, :, 256:512])
        nc.sync.dma_start(wq_sb[:, :, 0:128], wq_r[:, :, 0:128])
        nc.sync.dma_start(wv_sb[:, :, 0:128], wv_r[:, :, 0:128])
        for hb in range(2, 2 * TB):
            nc.sync.dma_start(xt_sb[:, :, hb * 256:(hb + 1) * 256],
                              xt_r[:, :, hb * 256:(hb + 1) * 256])
        nc.sync.dma_start(wk_sb[:, :, 128:], wk_r[:, :, 128:])
        nc.sync.dma_start(wq_sb[:, :, 128:], wq_r[:, :, 128:])
        nc.sync.dma_start(wv_sb[:, :, 128:], wv_r[:, :, 128:])
        nc.sync.dma_start(id_sb[:], ident.ap())
        nc.sync.dma_start(wp_sb[:], wp_r[:])

        for _rep in range(reps):
            def proj_chunks(hp, qT, kT, vA):
                """Generator: project k, v, q of head-pair hp in ~1us chunks.

                Chunk order is just-in-time for the first attention sweep
                (qb0 over kv 0..15): k(tb0) and q(tb0) first so scores can
                start immediately, then v in kv order interleaved with the
                remaining k blocks, then the remaining q blocks.
                """
                fsl = slice(hp * 128, (hp + 1) * 128)

                def kq(w_sb, dstT, tb, nm, half):
                    tok = slice(tb * 512 + half * 256,
                                tb * 512 + (half + 1) * 256)
                    p = ps_proj.tile([128, 256], F32, tag="proj",
                                     name=f"ps{nm}{hp}_{tb}_{half}")
                    for ko in range(KO):
                        nc.tensor.matmul(p[:], w_sb[:, ko, fsl],
                                         xt_sb[:, ko, tok],
                                         start=(ko == 0), stop=(ko == KO - 1))
                    nc.vector.tensor_copy(dstT[:, tok], p[:])

                def v_tile(tt):
                    tok = slice(tt * 128, (tt + 1) * 128)
                    psv = ps_proj.tile([128, 128], F32, tag="proj",
                                       name=f"psv{hp}_{tt}")
                    for ko in range(KO):
                        nc.tensor.matmul(psv[:], xt_sb[:, ko, tok],
                                         wv_sb[:, ko, fsl],
                                         start=(ko == 0), stop=(ko == KO - 1))
                    dst = vA[:, tt, :].rearrange("p (l c) -> p l c", l=2)
                    src = psv.rearrange("p (l c) -> p l c", l=2)
                    nc.vector.tensor_copy(dst[:, :, 0:64], src[:])

                for half in (0, 1):
                    kq(wk_sb, kT, 0, "k", half)
                    yield
                for half in (0, 1):
                    kq(wq_sb, qT, 0, "q", half)
                    yield
                nc.vector.memset(vA[:, :, 64], 1.0)
                nc.vector.memset(vA[:, :, 129], 1.0)
                for grp in range(TB):
                    if grp > 0:
                        for half in (0, 1):
                            kq(wk_sb, kT, grp, "k", half)
                            yield
                    for tt in range(grp * 4, grp * 4 + 4):
                        v_tile(tt)
                        yield
                for tb in range(1, TB):
                    for half in (0, 1):
                        kq(wq_sb, qT, tb, "q", half)
                        yield

            def attn_steps(hp, qT, kT, vA):
                """Generator: attention for head-pair hp, one kv step or one
                epilogue per yield.

                The PV accumulators hold 4 q-subtile chains per PSUM bank.
                A matmul with start=True lazily zeroes its bank's ENTIRE 2KB
                zero region, so interleaved chains in one bank would wipe
                each other's partials (and the scheduler is free to
                interleave disjoint-region writers).  Instead the tiles are
                zeroed once with a DVE memset and every PV matmul
                accumulates (start=False, group check off) -- adds commute,
                so any execution order is correct.
                """
                for qb in range(QB):
                    qsl = slice(qb * 512, (qb + 1) * 512)
                    accs = [ps_acc.tile([128, 4, 65], F32, tag=f"acc{h}",
                                        name=f"acc{h}_{hp}_{qb}")
                            for h in (0, 1)]
                    for h in (0, 1):
                        nc.vector.memset(accs[h][:], 0.0)
                    for kv in range(KV):
                        ksl = slice(kv * 128, (kv + 1) * 128)
                        st = ps_st.tile([128, 2, 512], F32, tag="st",
                                        name=f"st{hp}_{qb}_{kv}")
                        for h in (0, 1):
                            hsl = slice(h * 64, (h + 1) * 64)
                            nc.tensor.matmul(st[:, h, :], kT[hsl, ksl],
                                             qT[hsl, qsl],
                                             start=True, stop=True)
                        off = SCH_KV.get(kv)
                        if off is None:
                            pt = pt_p.tile([128, 2, 512], BF16, tag="pt",
                                           name=f"pt{hp}_{qb}_{kv}")
                            nc.scalar.activation(
                                pt[:], st[:],
                                mybir.ActivationFunctionType.Exp)
                        else:
                            pti = pt_p.tile([128, 2, 512], I16, tag="pti",
                                            bufs=4, name=f"pti{hp}_{qb}_{kv}")
                            nc.vector.tensor_scalar(pti[:], st[:],
                                                    SCH_A, SCH_B,
                                                    mybir.AluOpType.mult,
                                                    mybir.AluOpType.add)
                            pt = pti.bitcast(BF16)
                        # Deprioritize PV: when both are ready the PE should
                        # run the ACT-feeding score/proj work first; the pt
                        # pool gives PV ~PT_BUFS steps of laxity and pt-slot
                        # pressure self-balances.
                        po = tc.cur_priority
                        tc.cur_priority = po + PV_PRIO_OFFSET
                        for h in (0, 1):
                            vsl = slice(h * 65, (h + 1) * 65)
                            for sub in range(4):
                                nc.tensor.matmul(
                                    accs[h][:, sub, :],
                                    pt[:, h, sub * 128:(sub + 1) * 128],
                                    vA[:, kv, vsl],
                                    start=False, stop=(kv == KV - 1),
                                    skip_group_check=True)
                        tc.cur_priority = po
                        yield
                    # Epilogue: normalize o = num/den on the PSUM->SBUF
                    # copy, then transpose to oT[qb][:, hp, :].
                    rden = rden_p.tile([128, 2, 4], F32, tag="rden",
                                       name=f"rden{hp}_{qb}")
                    onrm = onorm_p.tile([128, 4, 2, 64], BF16, tag="onorm",
                                        name=f"onrm{hp}_{qb}")
                    nc.vector.reciprocal(rden[:, 0, :], accs[0][:, :, 64])
                    for sub in range(4):
                        nc.vector.tensor_scalar_mul(
                            onrm[:, sub, 0, :], accs[0][:, sub, 0:64],
                            rden[:, 0, sub:sub + 1])
                    yield
                    nc.vector.reciprocal(rden[:, 1, :], accs[1][:, :, 64])
                    for sub in range(4):
                        nc.vector.tensor_scalar_mul(
                            onrm[:, sub, 1, :], accs[1][:, sub, 0:64],
                            rden[:, 1, sub:sub + 1])
                    tp = ps_proj.tile([128, 4, 128], BF16, tag="proj",
                                      name=f"tp{hp}_{qb}")
                    for sub in range(4):
                        nc.tensor.matmul(tp[:, sub, :],
                                         onrm[:, sub, :, :], id_sb[:],
                                         is_transpose=True,
                                         start=(sub == 0), stop=(sub == 3),
                                         skip_group_check=True)
                    nc.vector.tensor_copy(
                        oT[qb][:, hp, :],
                        tp.rearrange("p a b -> p (a b)"))
                    yield

            def oproj_steps(qb):
                """Generator: partial out[qsl, :] = oT_qb^T @ w_proj_local,
                one [128 tok, 256 cout] PSUM tenure per yield."""
                for tt in range(4):
                    tsl = slice(tt * 128, (tt + 1) * 128)
                    osl = slice(qb * 512 + tt * 128, qb * 512 + (tt + 1) * 128)
                    for co in range(4):
                        csl = slice(co * 256, (co + 1) * 256)
                        ps = ps_proj.tile([128, 256], F32, tag="proj",
                                          name=f"op{qb}_{tt}_{co}")
                        for ko in range(KO2):
                            nc.tensor.matmul(ps[:], oT[qb][:, ko, tsl],
                                             wp_sb[:, ko, csl],
                                             start=(ko == 0),
                                             stop=(ko == KO2 - 1))
                        ob = outp.tile([128, 256], F32, tag="o",
                                       name=f"ob{qb}_{tt}_{co}")
                        nc.vector.tensor_copy(ob[:], ps[:])
                        nc.sync.dma_start(out.ap()[osl, csl], ob[:])
                        yield

            def hp_tiles(hp):
                qT = qk_p.tile([128, N], BF16, tag="qT", name=f"qT{hp}")
                kT = qk_p.tile([128, N], BF16, tag="kT", name=f"kT{hp}")
                vA = v_p.tile([128, KV, 130], BF16, tag="vA", name=f"vA{hp}")
                return qT, kT, vA

            # Interleave projection chunks (~1 per attention step) with the
            # attention steps; the tile scheduler resolves real deps, the
            # emission order sets priorities.  Attention(hp0) is emitted
            # right after k(tb0)+q(tb0) so the ACT pipeline starts ~10us in.
            # Emission order IS dependency order for the tile framework: an
            # attention step must be emitted AFTER the proj chunks it reads.
            # need_chunks[step] = how many chunks of the CURRENT head-pair's
            # generator must be emitted before attention step `step` (chunk
            # order: k0 q0 v0-3 k1 v4-7 k2 v8-11 k3 v12-15 q1 q2 q3).
            vpos = [4, 5, 6, 7, 10, 11, 12, 13, 16, 17, 18, 19, 22, 23, 24, 25]

            def need_chunks(step):
                qb, within = divmod(step, KV + 2)
                if qb == 0:
                    return vpos[min(within, KV - 1)] + 1
                return 26 + 2 * min(qb, 3)

            from collections import deque
            cur = hp_tiles(0)
            pending = deque([[0, proj_chunks(0, *cur), 0]])

            def pull_one():
                while pending:
                    ent = pending[0]
                    if next(ent[1], StopIteration) is StopIteration:
                        pending.popleft()
                    else:
                        ent[2] += 1
                        return
            for hp in range(NHP := HL // 2):
                if hp < NHP - 1:
                    nxt = hp_tiles(hp + 1)
                    pending.append([hp + 1, proj_chunks(hp + 1, *nxt), 0])
                else:
                    nxt = None
                agen = attn_steps(hp, *cur)
                for step in range(QB * (KV + 2)):
                    # hard requirement: current head-pair's chunks this
                    # attention step reads must already be emitted
                    while (pending and pending[0][0] == hp
                           and pending[0][2] < need_chunks(step)):
                        pull_one()
                    # cadence fill: one chunk every 3rd step regardless of
                    # owner -- the need-driven pulls above already force
                    # everything an attention step reads, so eager draining
                    # only crowds the PE and slips ACT.  During the last
                    # head-pair the queue holds oproj tenures and there are
                    # no more ACT-feeding proj chunks, so drain every step.
                    if pending and (step % 3 == 0 or hp == NHP - 1):
                        pull_one()
                    next(agen, None)
                    if hp == NHP - 1 and step % (KV + 2) == KV + 1:
                        qb_done = step // (KV + 2)
                        pending.append([-1, oproj_steps(qb_done), 0])
                cur = nxt
            while pending:
                if next(pending[0][1], StopIteration) is StopIteration:
                    pending.popleft()

    nc.compile()
    return nc


# ---------------------------------------------------------------- runner
class _SpmdRunner:
    """jit-once SPMD runner over n cores (modeled on bass2jax.run_bass_via_pjrt)."""

    def __init__(self, nc, n_cores):
        import jax
        from jax.experimental.shard_map import shard_map
        from jax.sharding import Mesh, PartitionSpec
        from concourse.bass2jax import (_bass_exec_p, install_neuronx_cc_hook,
                                        partition_id_tensor)

        install_neuronx_cc_hook()
        self.jax = jax
        self.n_cores = n_cores
        partition_name = (nc.partition_id_tensor.name
                          if nc.partition_id_tensor else None)
        in_names, out_names, out_avals, zero_shapes = [], [], [], []
        for alloc in nc.m.functions[0].allocations:
            if not isinstance(alloc, mybir.MemoryLocationSet):
                continue
            name = alloc.memorylocations[0].name
            if alloc.kind == "ExternalInput":
                if name != partition_name:
                    in_names.append(name)
            elif alloc.kind == "ExternalOutput":
                shape = tuple(alloc.tensor_shape)
                dtype = mybir.dt.np(alloc.dtype)
                out_names.append(name)
                out_avals.append(jax.core.ShapedArray(shape, dtype))
                zero_shapes.append((shape, dtype))
        self.in_names, self.out_names = in_names, out_names
        self.out_avals, self.zero_shapes = out_avals, zero_shapes
        n_params, n_outs = len(in_names), len(out_names)
        all_in = list(in_names) + list(out_names)
        if partition_name is not None:
            all_in.append(partition_name)

        def _body(*args):
            operands = list(args)
            if partition_name is not None:
                operands.append(partition_id_tensor())
            return tuple(_bass_exec_p.bind(
                *operands, out_avals=tuple(out_avals), in_names=tuple(all_in),
                out_names=tuple(out_names), lowering_input_output_aliases=(),
                sim_require_finite=True, sim_require_nnan=True, nc=nc))

        devices = jax.devices()[:n_cores]
        self.mesh = Mesh(np.asarray(devices), ("core",))
        self.pspec = PartitionSpec("core")
        in_specs = (self.pspec,) * (n_params + n_outs)
        out_specs = (self.pspec,) * n_outs
        self.fn = jax.jit(
            shard_map(_body, mesh=self.mesh, in_specs=in_specs,
                      out_specs=out_specs, check_rep=False),
            donate_argnums=tuple(range(n_params, n_params + n_outs)),
            keep_unused=True)

    def _stage(self, in_maps):
        from jax.sharding import NamedSharding
        sharding = NamedSharding(self.mesh, self.pspec)
        concat = [np.concatenate([np.asarray(m[n]) for m in in_maps], axis=0)
                  for n in self.in_names]
        dev_in = [self.jax.device_put(x, sharding) for x in concat]
        for x in dev_in:
            x.block_until_ready()
        return sharding, dev_in

    def _zeros(self, sharding):
        zeros = [self.jax.device_put(
            np.zeros((self.n_cores * s[0], *s[1:]), d), sharding)
            for (s, d) in self.zero_shapes]
        for z in zeros:
            z.block_until_ready()
        return zeros

    def _unpack(self, outs):
        np_outs = [np.asarray(o) for o in outs]
        return [
            {n: np_outs[i].reshape(self.n_cores, *self.out_avals[i].shape)[c]
             for i, n in enumerate(self.out_names)}
            for c in range(self.n_cores)
        ]

    def run(self, in_maps):
        sharding, dev_in = self._stage(in_maps)
        outs = self.fn(*dev_in, *self._zeros(sharding))
        return self._unpack(outs)

    def timed_run(self, in_maps, iters=6):
        """Stage inputs once; time only execute+sync per iteration."""
        import time
        sharding, dev_in = self._stage(in_maps)
        walls = []
        outs = None
        for _ in range(iters):
            zeros = self._zeros(sharding)
            t0 = time.perf_counter()
            outs = self.fn(*dev_in, *zeros)
            for o in outs:
                o.block_until_ready()
            walls.append(time.perf_counter() - t0)
        return self._unpack(outs), walls


_STATE = {}


def _get_state():
    if "l1" not in _STATE:
        nc1 = _build_l1()
        _STATE["l1"] = nc1
        _STATE["r1"] = _SpmdRunner(nc1, NCORES)
    return _STATE


def _l1_in_maps(x, w_qkv, w_proj):
    scale = np.float32(D ** -0.5)
    ident = np.eye(128, dtype=NP_BF16)
    in_maps = []
    for c in range(NCORES):
        b = c // 2
        hg = c % 2
        fsl = slice(hg * FL, (hg + 1) * FL)
        in_maps.append({
            "xt": np.ascontiguousarray(x[b].T).astype(NP_BF16),
            "wq": (np.ascontiguousarray(w_qkv[:, fsl]) * scale).astype(NP_BF16),
            "wk": np.ascontiguousarray(w_qkv[:, C:][:, fsl]).astype(NP_BF16),
            "wv": np.ascontiguousarray(w_qkv[:, 2 * C:][:, fsl]).astype(NP_BF16),
            "wp": np.ascontiguousarray(w_proj[fsl, :]).astype(NP_BF16),
            "ident": ident,
        })
    return in_maps


def kernel(x, w_qkv, w_proj, b_proj):
    st = _get_state()
    x = np.asarray(x, dtype=np.float32)
    w_qkv = np.asarray(w_qkv, dtype=np.float32)
    w_proj = np.asarray(w_proj, dtype=np.float32)
    b_proj = np.asarray(b_proj, dtype=np.float32)

    res = st["r1"].run(_l1_in_maps(x, w_qkv, w_proj))

    # host: sum the two head-group partials per batch, add bias
    out = np.empty((B, N, C), dtype=np.float32)
    for b in range(B):
        out[b] = res[2 * b]["out"] + res[2 * b + 1]["out"] + b_proj
    return out


# revision 11
# speedup vs baseline: 1.0131x; 1.0007x over previous
"""Multi-head attention forward on 8 Trainium2 NeuronCores (Bass/Tile).

Problem: B=4, N=2048, C=1024, H=16, D=64.
    qkv = x @ w_qkv ; per-head scaled softmax(q k^T) v ; o @ w_proj + b_proj

Sharding: core c handles batch (c // 2) and heads (c % 2)*8 .. +8.
ONE SPMD launch per core (the old separate o@w_proj launch is merged in):

  - per-core qkv projection + flash-style attention over its 8 heads
    (4 head-pairs) of its batch.  All matmul operands are bf16.  Structure
    per head-pair hp:
      - project k, v, q for hp (PE; interleaved in program order with
        the attention of head-pair hp-1),
      - attention: per (qb, kv): S^T = k^T q (2 matmuls, K=64, M=128,
        F=512 into one [128,2,512] PSUM tile), exp on ACT ([128,1024]
        per instruction, PSUM -> SBUF bf16), then PV with the exp
        output as lhsT: out[q=128, 65] += P V_aug (K=128, M=128, F=65).
        V is augmented with a ones column so row sums (softmax
        denominators) fall out of the same matmuls.
  - Epilogue per (qb, hp): DVE reciprocal of the denominator column,
    normalize-on-copy (PSUM fp32 -> SBUF bf16, tensor_scalar multiply
    by 1/den per q-subtile), then 4 PE transposes (identity matmul,
    one shared-pool PSUM bank) build oT[qb][:, hp, :] = o_norm^T.
  - After head-pair 3 finishes q-block qb: 16 output-projection
    tenures out_partial[qsl, :] = oT_qb^T(512 local feats) @ w_proj
    rotate through the same 2-bank PSUM proj pool.  DMA out fp32.
  (host) out[b] = partial[2b] + partial[2b+1] + b_proj  (free).

  PSUM budget: st 2x2 banks + acc 2x1 + proj/transpose/oproj pool 2x1
  = 8 banks exactly.

Cost model per core: PE = proj 196608 + scores 262144 + PV 133120
+ transposes 8192 + oproj 65536 = 665600 cycles = 277.3us @2.4GHz (the
bound); ACT exp 256 x 1.04us = 265.7us just fits under it.  The merge
removes the old launch-2 (40.1us) entirely.
"""

import numpy as np

import concourse.bacc as bacc
import concourse.bass as bass
import concourse.tile as tile
from concourse import mybir

F32 = mybir.dt.float32
BF16 = mybir.dt.bfloat16
I16 = mybir.dt.int16
NP_BF16 = mybir.dt.np(mybir.dt.bfloat16)

# Schraudolph exp in bf16-bit domain: bf16_bits(exp(s)) ~= s*128/ln2 + 127*128
# + corr.  Computed as int16 on the DVE for 2 of the 16 kv steps per sweep,
# relieving the ACT exp bottleneck (GPSIMD cannot access PSUM, so the DVE
# takes both).  corr = -7 minimizes rms rel err (1.8%); +0.5 compensates the
# truncating float->int16 convert.  rel-err impact at 2/16 offload: ~+1e-3.
SCH_A = float(128.0 / np.log(2.0))
SCH_B = float(127 * 128 - 7 + 0.5)
SCH_KV = {5: "dve", 11: "dve"}

B, N, C, H = 4, 2048, 1024, 16
D = C // H          # 64
NCORES = 8
HL = H // 2         # 8 local heads per core
FL = HL * D         # 512 local features
KO = C // 128       # 8 contraction tiles (qkv proj)
KO2 = FL // 128     # 4 contraction tiles (out proj)
TB = N // 512       # 4 token blocks of 512
KV = N // 128       # 16 kv tiles of 128
QB = N // 512       # 4 query blocks of 512


# tuning knobs (timeline-sim sweeps)
PT_BUFS = 10
ST_BUFS = 2
PROJ_BUFS = 2
PV_PRIO_OFFSET = 60


# ---------------------------------------------------------------- launch
def _build_l1(reps=1):
    nc = bacc.Bacc("TRN2", target_bir_lowering=False, debug=False)
    xt = nc.dram_tensor("xt", [C, N], BF16, kind="ExternalInput")
    wq = nc.dram_tensor("wq", [C, FL], BF16, kind="ExternalInput")
    wk = nc.dram_tensor("wk", [C, FL], BF16, kind="ExternalInput")
    wv = nc.dram_tensor("wv", [C, FL], BF16, kind="ExternalInput")
    wp = nc.dram_tensor("wp", [FL, C], BF16, kind="ExternalInput")
    ident = nc.dram_tensor("ident", [128, 128], BF16, kind="ExternalInput")
    out = nc.dram_tensor("out", [N, C], F32, kind="ExternalOutput")

    xt_r = xt.ap().rearrange("(o p) n -> p o n", p=128)
    wq_r = wq.ap().rearrange("(o p) f -> p o f", p=128)
    wk_r = wk.ap().rearrange("(o p) f -> p o f", p=128)
    wv_r = wv.ap().rearrange("(o p) f -> p o f", p=128)
    wp_r = wp.ap().rearrange("(o p) c -> p o c", p=128)

    with (
        tile.TileContext(nc) as tc,
        tc.tile_pool(name="persist", bufs=1) as persist,
        tc.tile_pool(name="qk", bufs=2) as qk_p,
        tc.tile_pool(name="vp", bufs=2) as v_p,
        tc.tile_pool(name="pt", bufs=PT_BUFS) as pt_p,
        tc.tile_pool(name="onorm", bufs=4) as onorm_p,
        tc.tile_pool(name="rden", bufs=2) as rden_p,
        tc.tile_pool(name="outp", bufs=4) as outp,
        tc.tile_pool(name="ps_proj", bufs=PROJ_BUFS, space="PSUM") as ps_proj,
        tc.tile_pool(name="ps_st", bufs=ST_BUFS, space="PSUM") as ps_st,
        tc.tile_pool(name="ps_acc", bufs=1, space="PSUM") as ps_acc,
    ):
        xt_sb = persist.tile([128, KO, N], BF16)
        wq_sb = persist.tile([128, KO, FL], BF16)
        wk_sb = persist.tile([128, KO, FL], BF16)
        wv_sb = persist.tile([128, KO, FL], BF16)
        wp_sb = persist.tile([128, KO2, C], BF16)
        id_sb = persist.tile([128, 128], BF16)
        # oT[qb]: o_norm^T [512 local feats (4 ko-chunks of 128), 512 toks]
        oT = [persist.tile([128, KO2, 512], BF16, name=f"oT{qb}")
              for qb in range(QB)]
        # All loads on the SP queue in just-in-time order for the first
        # attention sweep (transfers serialize on the shared DMA engines, so
        # the queue order IS the arrival order): the head-pair-0 column
        # slices of the weights (cheap 128-col loads) + xt(tb0) enable the
        # first k/q/v chunks by ~6us, xt(tb1..3) arrive right before the
        # k(tb1..3) chunks need them, and the remaining weight columns
        # trail in (first needed by head-pair 1, ~70us later).  w_proj and
        # the transpose identity are first needed at the hp0-qb0 epilogue
        # (~80us in), so they ride at the back.
        nc.sync.dma_start(wk_sb[:, :, 0:128], wk_r[:, :, 0:128])
        nc.sync.dma_start(xt_sb[:, :, 0:256], xt_r[:, :, 0:256])
        nc.sync.dma_start(xt_sb[:, :, 256:512], xt_r[:, :, 256:512])
        nc.sync.dma_start(wq_sb[:, :, 0:128], wq_r[:, :, 0:128])
        nc.sync.dma_start(wv_sb[:, :, 0:128], wv_r[:, :, 0:128])
        for hb in range(2, 2 * TB):
            nc.sync.dma_start(xt_sb[:, :, hb * 256:(hb + 1) * 256],
                              xt_r[:, :, hb * 256:(hb + 1) * 256])
        nc.sync.dma_start(wk_sb[:, :, 128:], wk_r[:, :, 128:])
        nc.sync.dma_start(wq_sb[:, :, 128:], wq_r[:, :, 128:])
        nc.sync.dma_start(wv_sb[:, :, 128:], wv_r[:, :, 128:])
        nc.sync.dma_start(id_sb[:], ident.ap())
        nc.sync.dma_start(wp_sb[:], wp_r[:])

        for _rep in range(reps):
            def proj_chunks(hp, qT, kT, vA):
                """Generator: project k, v, q of head-pair hp in ~1us chunks.

                Chunk order is just-in-time for the first attention sweep
                (qb0 over kv 0..15): k(tb0) and q(tb0) first so scores can
                start immediately, then v in kv order interleaved with the
                remaining k blocks, then the remaining q blocks.
                """
                fsl = slice(hp * 128, (hp + 1) * 128)

                def kq(w_sb, dstT, tb, nm, half):
                    tok = slice(tb * 512 + half * 256,
                                tb * 512 + (half + 1) * 256)
                    p = ps_proj.tile([128, 256], F32, tag="proj",
                                     name=f"ps{nm}{hp}_{tb}_{half}")
                    for ko in range(KO):
                        nc.tensor.matmul(p[:], w_sb[:, ko, fsl],
                                         xt_sb[:, ko, tok],
                                         start=(ko == 0), stop=(ko == KO - 1))
                    nc.vector.tensor_copy(dstT[:, tok], p[:])

                def v_tile(tt):
                    tok = slice(tt * 128, (tt + 1) * 128)
                    psv = ps_proj.tile([128, 128], F32, tag="proj",
                                       name=f"psv{hp}_{tt}")
                    for ko in range(KO):
                        nc.tensor.matmul(psv[:], xt_sb[:, ko, tok],
                                         wv_sb[:, ko, fsl],
                                         start=(ko == 0), stop=(ko == KO - 1))
                    dst = vA[:, tt, :].rearrange("p (l c) -> p l c", l=2)
                    src = psv.rearrange("p (l c) -> p l c", l=2)
                    nc.vector.tensor_copy(dst[:, :, 0:64], src[:])

                for half in (0, 1):
                    kq(wk_sb, kT, 0, "k", half)
                    yield
                for half in (0, 1):
                    kq(wq_sb, qT, 0, "q", half)
                    yield
                nc.vector.memset(vA[:, :, 64], 1.0)
                nc.vector.memset(vA[:, :, 129], 1.0)
                for grp in range(TB):
                    if grp > 0:
                        for half in (0, 1):
                            kq(wk_sb, kT, grp, "k", half)
                            yield
                    for tt in range(grp * 4, grp * 4 + 4):
                        v_tile(tt)
                        yield
                for tb in range(1, TB):
                    for half in (0, 1):
                        kq(wq_sb, qT, tb, "q", half)
                        yield

            def attn_steps(hp, qT, kT, vA):
                """Generator: attention for head-pair hp, one kv step or one
                epilogue per yield.

                The PV accumulators hold 4 q-subtile chains per PSUM bank.
                A matmul with start=True lazily zeroes its bank's ENTIRE 2KB
                zero region; each acc tile owns its whole bank, and the PE
                executes its queue in order, so the FIRST PV matmul of the
                bank (kv0, sub0) zeroes it with start=True and every other
                PV matmul accumulates (start=False, group check off) --
                adds commute, so any execution order of the disjoint
                sub-chains is correct.  (A DVE memset would ride behind the
                previous q-block's epilogue in DVE program order and stall
                the first PV chains ~2us at every q-block boundary.)
                """
                for qb in range(QB):
                    qsl = slice(qb * 512, (qb + 1) * 512)
                    accs = [ps_acc.tile([128, 4, 65], F32, tag=f"acc{h}",
                                        name=f"acc{h}_{hp}_{qb}")
                            for h in (0, 1)]
                    for kv in range(KV):
                        ksl = slice(kv * 128, (kv + 1) * 128)
                        st = ps_st.tile([128, 2, 512], F32, tag="st",
                                        name=f"st{hp}_{qb}_{kv}")
                        for h in (0, 1):
                            hsl = slice(h * 64, (h + 1) * 64)
                            nc.tensor.matmul(st[:, h, :], kT[hsl, ksl],
                                             qT[hsl, qsl],
                                             start=True, stop=True)
                        off = SCH_KV.get(kv)
                        if off is None:
                            pt = pt_p.tile([128, 2, 512], BF16, tag="pt",
                                           name=f"pt{hp}_{qb}_{kv}")
                            nc.scalar.activation(
                                pt[:], st[:],
                                mybir.ActivationFunctionType.Exp)
                        else:
                            pti = pt_p.tile([128, 2, 512], I16, tag="pti",
                                            bufs=4, name=f"pti{hp}_{qb}_{kv}")
                            nc.vector.tensor_scalar(pti[:], st[:],
                                                    SCH_A, SCH_B,
                                                    mybir.AluOpType.mult,
                                                    mybir.AluOpType.add)
                            pt = pti.bitcast(BF16)
                        # Deprioritize PV: when both are ready the PE should
                        # run the ACT-feeding score/proj work first; the pt
                        # pool gives PV ~PT_BUFS steps of laxity and pt-slot
                        # pressure self-balances.
                        po = tc.cur_priority
                        tc.cur_priority = po + PV_PRIO_OFFSET
                        for h in (0, 1):
                            vsl = slice(h * 65, (h + 1) * 65)
                            for sub in range(4):
                                nc.tensor.matmul(
                                    accs[h][:, sub, :],
                                    pt[:, h, sub * 128:(sub + 1) * 128],
                                    vA[:, kv, vsl],
                                    start=(kv == 0 and sub == 0),
                                    stop=(kv == KV - 1),
                                    skip_group_check=True)
                        tc.cur_priority = po
                        yield
                    # Epilogue: normalize o = num/den on the PSUM->SBUF
                    # copy, then transpose to oT[qb][:, hp, :].
                    rden = rden_p.tile([128, 2, 4], F32, tag="rden",
                                       name=f"rden{hp}_{qb}")
                    onrm = onorm_p.tile([128, 4, 2, 64], BF16, tag="onorm",
                                        name=f"onrm{hp}_{qb}")
                    nc.vector.reciprocal(rden[:, 0, :], accs[0][:, :, 64])
                    for sub in range(4):
                        nc.vector.tensor_scalar_mul(
                            onrm[:, sub, 0, :], accs[0][:, sub, 0:64],
                            rden[:, 0, sub:sub + 1])
                    yield
                    nc.vector.reciprocal(rden[:, 1, :], accs[1][:, :, 64])
                    for sub in range(4):
                        nc.vector.tensor_scalar_mul(
                            onrm[:, sub, 1, :], accs[1][:, sub, 0:64],
                            rden[:, 1, sub:sub + 1])
                    # The transposes (PE) depend on the DVE normalize chain;
                    # emitting them here would park them at the head of the
                    # PE's in-order queue and stall the next q-block's
                    # scores.  Defer them to the pending work queue instead
                    # (pulled a step or two into the next sweep).
                    pending.append([-1, transpose_steps(hp, qb, onrm), 0])
                    yield

            def transpose_steps(hp, qb, onrm):
                tp = ps_proj.tile([128, 4, 128], BF16, tag="proj",
                                  name=f"tp{hp}_{qb}")
                for sub in range(4):
                    nc.tensor.matmul(tp[:, sub, :],
                                     onrm[:, sub, :, :], id_sb[:],
                                     is_transpose=True,
                                     start=(sub == 0), stop=(sub == 3),
                                     skip_group_check=True)
                nc.vector.tensor_copy(
                    oT[qb][:, hp, :],
                    tp.rearrange("p a b -> p (a b)"))
                yield

            def oproj_steps(qb):
                """Generator: partial out[qsl, :] = oT_qb^T @ w_proj_local,
                one [128 tok, 256 cout] PSUM tenure per yield."""
                for tt in range(4):
                    tsl = slice(tt * 128, (tt + 1) * 128)
                    osl = slice(qb * 512 + tt * 128, qb * 512 + (tt + 1) * 128)
                    for co in range(4):
                        csl = slice(co * 256, (co + 1) * 256)
                        ps = ps_proj.tile([128, 256], F32, tag="proj",
                                          name=f"op{qb}_{tt}_{co}")
                        for ko in range(KO2):
                            nc.tensor.matmul(ps[:], oT[qb][:, ko, tsl],
                                             wp_sb[:, ko, csl],
                                             start=(ko == 0),
                                             stop=(ko == KO2 - 1))
                        ob = outp.tile([128, 256], F32, tag="o",
                                       name=f"ob{qb}_{tt}_{co}")
                        nc.vector.tensor_copy(ob[:], ps[:])
                        nc.sync.dma_start(out.ap()[osl, csl], ob[:])
                        yield

            def hp_tiles(hp):
                qT = qk_p.tile([128, N], BF16, tag="qT", name=f"qT{hp}")
                kT = qk_p.tile([128, N], BF16, tag="kT", name=f"kT{hp}")
                vA = v_p.tile([128, KV, 130], BF16, tag="vA", name=f"vA{hp}")
                return qT, kT, vA

            # Interleave projection chunks (~1 per attention step) with the
            # attention steps; the tile scheduler resolves real deps, the
            # emission order sets priorities.  Attention(hp0) is emitted
            # right after k(tb0)+q(tb0) so the ACT pipeline starts ~10us in.
            # Emission order IS dependency order for the tile framework: an
            # attention step must be emitted AFTER the proj chunks it reads.
            # need_chunks[step] = how many chunks of the CURRENT head-pair's
            # generator must be emitted before attention step `step` (chunk
            # order: k0 q0 v0-3 k1 v4-7 k2 v8-11 k3 v12-15 q1 q2 q3).
            vpos = [4, 5, 6, 7, 10, 11, 12, 13, 16, 17, 18, 19, 22, 23, 24, 25]

            def need_chunks(step):
                qb, within = divmod(step, KV + 2)
                if qb == 0:
                    return vpos[min(within, KV - 1)] + 1
                return 26 + 2 * min(qb, 3)

            from collections import deque
            cur = hp_tiles(0)
            pending = deque([[0, proj_chunks(0, *cur), 0]])

            def pull_one():
                while pending:
                    ent = pending[0]
                    if next(ent[1], StopIteration) is StopIteration:
                        pending.popleft()
                    else:
                        ent[2] += 1
                        return
            for hp in range(NHP := HL // 2):
                if hp < NHP - 1:
                    nxt = hp_tiles(hp + 1)
                    pending.append([hp + 1, proj_chunks(hp + 1, *nxt), 0])
                else:
                    nxt = None
                agen = attn_steps(hp, *cur)
                for step in range(QB * (KV + 2)):
                    # hard requirement: current head-pair's chunks this
                    # attention step reads must already be emitted
                    while (pending and pending[0][0] == hp
                           and pending[0][2] < need_chunks(step)):
                        pull_one()
                    # cadence fill: one chunk every 3rd step regardless of
                    # owner -- the need-driven pulls above already force
                    # everything an attention step reads, so eager draining
                    # only crowds the PE and slips ACT.  During the last
                    # head-pair the queue holds oproj tenures and there are
                    # no more ACT-feeding proj chunks, so drain every step.
                    if pending and (step % 3 == 0 or hp == NHP - 1):
                        pull_one()
                    next(agen, None)
                    if hp == NHP - 1 and step % (KV + 2) == KV + 1:
                        qb_done = step // (KV + 2)
                        pending.append([-1, oproj_steps(qb_done), 0])
                cur = nxt
            while pending:
                if next(pending[0][1], StopIteration) is StopIteration:
                    pending.popleft()

    nc.compile()
    return nc


# ---------------------------------------------------------------- runner
class _SpmdRunner:
    """jit-once SPMD runner over n cores (modeled on bass2jax.run_bass_via_pjrt)."""

    def __init__(self, nc, n_cores):
        import jax
        from jax.experimental.shard_map import shard_map
        from jax.sharding import Mesh, PartitionSpec
        from concourse.bass2jax import (_bass_exec_p, install_neuronx_cc_hook,
                                        partition_id_tensor)

        install_neuronx_cc_hook()
        self.jax = jax
        self.n_cores = n_cores
        partition_name = (nc.partition_id_tensor.name
                          if nc.partition_id_tensor else None)
        in_names, out_names, out_avals, zero_shapes = [], [], [], []
        for alloc in nc.m.functions[0].allocations:
            if not isinstance(alloc, mybir.MemoryLocationSet):
                continue
            name = alloc.memorylocations[0].name
            if alloc.kind == "ExternalInput":
                if name != partition_name:
                    in_names.append(name)
            elif alloc.kind == "ExternalOutput":
                shape = tuple(alloc.tensor_shape)
                dtype = mybir.dt.np(alloc.dtype)
                out_names.append(name)
                out_avals.append(jax.core.ShapedArray(shape, dtype))
                zero_shapes.append((shape, dtype))
        self.in_names, self.out_names = in_names, out_names
        self.out_avals, self.zero_shapes = out_avals, zero_shapes
        n_params, n_outs = len(in_names), len(out_names)
        all_in = list(in_names) + list(out_names)
        if partition_name is not None:
            all_in.append(partition_name)

        def _body(*args):
            operands = list(args)
            if partition_name is not None:
                operands.append(partition_id_tensor())
            return tuple(_bass_exec_p.bind(
                *operands, out_avals=tuple(out_avals), in_names=tuple(all_in),
                out_names=tuple(out_names), lowering_input_output_aliases=(),
                sim_require_finite=True, sim_require_nnan=True, nc=nc))

        devices = jax.devices()[:n_cores]
        self.mesh = Mesh(np.asarray(devices), ("core",))
        self.pspec = PartitionSpec("core")
        in_specs = (self.pspec,) * (n_params + n_outs)
        out_specs = (self.pspec,) * n_outs
        self.fn = jax.jit(
            shard_map(_body, mesh=self.mesh, in_specs=in_specs,
                      out_specs=out_specs, check_rep=False),
            donate_argnums=tuple(range(n_params, n_params + n_outs)),
            keep_unused=True)

    def _stage(self, in_maps):
        from jax.sharding import NamedSharding
        sharding = NamedSharding(self.mesh, self.pspec)
        concat = [np.concatenate([np.asarray(m[n]) for m in in_maps], axis=0)
                  for n in self.in_names]
        dev_in = [self.jax.device_put(x, sharding) for x in concat]
        for x in dev_in:
            x.block_until_ready()
        return sharding, dev_in

    def _zeros(self, sharding):
        zeros = [self.jax.device_put(
            np.zeros((self.n_cores * s[0], *s[1:]), d), sharding)
            for (s, d) in self.zero_shapes]
        for z in zeros:
            z.block_until_ready()
        return zeros

    def _unpack(self, outs):
        np_outs = [np.asarray(o) for o in outs]
        return [
            {n: np_outs[i].reshape(self.n_cores, *self.out_avals[i].shape)[c]
             for i, n in enumerate(self.out_names)}
            for c in range(self.n_cores)
        ]

    def run(self, in_maps):
        sharding, dev_in = self._stage(in_maps)
        outs = self.fn(*dev_in, *self._zeros(sharding))
        return self._unpack(outs)

    def timed_run(self, in_maps, iters=6):
        """Stage inputs once; time only execute+sync per iteration."""
        import time
        sharding, dev_in = self._stage(in_maps)
        walls = []
        outs = None
        for _ in range(iters):
            zeros = self._zeros(sharding)
            t0 = time.perf_counter()
            outs = self.fn(*dev_in, *zeros)
            for o in outs:
                o.block_until_ready()
            walls.append(time.perf_counter() - t0)
        return self._unpack(outs), walls


_STATE = {}


def _get_state():
    if "l1" not in _STATE:
        nc1 = _build_l1()
        _STATE["l1"] = nc1
        _STATE["r1"] = _SpmdRunner(nc1, NCORES)
    return _STATE


def _l1_in_maps(x, w_qkv, w_proj):
    scale = np.float32(D ** -0.5)
    ident = np.eye(128, dtype=NP_BF16)
    in_maps = []
    for c in range(NCORES):
        b = c // 2
        hg = c % 2
        fsl = slice(hg * FL, (hg + 1) * FL)
        in_maps.append({
            "xt": np.ascontiguousarray(x[b].T).astype(NP_BF16),
            "wq": (np.ascontiguousarray(w_qkv[:, fsl]) * scale).astype(NP_BF16),
            "wk": np.ascontiguousarray(w_qkv[:, C:][:, fsl]).astype(NP_BF16),
            "wv": np.ascontiguousarray(w_qkv[:, 2 * C:][:, fsl]).astype(NP_BF16),
            "wp": np.ascontiguousarray(w_proj[fsl, :]).astype(NP_BF16),
            "ident": ident,
        })
    return in_maps


def kernel(x, w_qkv, w_proj, b_proj):
    st = _get_state()
    x = np.asarray(x, dtype=np.float32)
    w_qkv = np.asarray(w_qkv, dtype=np.float32)
    w_proj = np.asarray(w_proj, dtype=np.float32)
    b_proj = np.asarray(b_proj, dtype=np.float32)

    res = st["r1"].run(_l1_in_maps(x, w_qkv, w_proj))

    # host: sum the two head-group partials per batch, add bias
    out = np.empty((B, N, C), dtype=np.float32)
    for b in range(B):
        out[b] = res[2 * b]["out"] + res[2 * b + 1]["out"] + b_proj
    return out


# revision 12
# speedup vs baseline: 1.1075x; 1.0932x over previous
"""Multi-head attention forward on 8 Trainium2 NeuronCores (Bass/Tile).

Problem: B=4, N=2048, C=1024, H=16, D=64.
    qkv = x @ w_qkv ; per-head scaled softmax(q k^T) v ; o @ w_proj + b_proj

Sharding: core c handles batch (c // 2) and heads (c % 2)*8 .. +8.
ONE SPMD launch per core (the old separate o@w_proj launch is merged in):

  - per-core qkv projection + flash-style attention over its 8 heads
    (4 head-pairs) of its batch.  All matmul operands are bf16.  Structure
    per head-pair hp:
      - project k, v, q for hp (PE; interleaved in program order with
        the attention of head-pair hp-1),
      - attention: per (qb, kv): S^T = k^T q (2 matmuls, K=64, M=128,
        F=512 into one [128,2,512] PSUM tile), exp on ACT ([128,1024]
        per instruction, PSUM -> SBUF bf16), then PV with the exp
        output as lhsT: out[q=128, 65] += P V_aug (K=128, M=128, F=65).
        V is augmented with a ones column so row sums (softmax
        denominators) fall out of the same matmuls.
  - Epilogue per (qb, hp): DVE reciprocal of the denominator column,
    normalize-on-copy (PSUM fp32 -> SBUF bf16, tensor_scalar multiply
    by 1/den per q-subtile), then 4 PE transposes (identity matmul,
    one shared-pool PSUM bank) build oT[qb][:, hp, :] = o_norm^T.
  - After head-pair 3 finishes q-block qb: 16 output-projection
    tenures out_partial[qsl, :] = oT_qb^T(512 local feats) @ w_proj
    rotate through the same 2-bank PSUM proj pool.  DMA out fp32.
  (host) out[b] = partial[2b] + partial[2b+1] + b_proj  (free).

  PSUM budget: st 2x2 banks + acc 2x1 + proj/transpose/oproj pool 2x1
  = 8 banks exactly.

Cost model per core: PE = proj 196608 + scores 262144 + PV 133120
+ transposes 8192 + oproj 65536 = 665600 cycles = 277.3us @2.4GHz (the
bound); ACT exp 256 x 1.04us = 265.7us just fits under it.  The merge
removes the old launch-2 (40.1us) entirely.
"""

import numpy as np

import concourse.bacc as bacc
import concourse.bass as bass
import concourse.tile as tile
from concourse import mybir

F32 = mybir.dt.float32
BF16 = mybir.dt.bfloat16
I16 = mybir.dt.int16
NP_BF16 = mybir.dt.np(mybir.dt.bfloat16)

# Schraudolph exp in bf16-bit domain: bf16_bits(exp(s)) ~= s*128/ln2 + 127*128
# + corr.  Computed as int16 on the DVE for 2 of the 16 kv steps per sweep,
# relieving the ACT exp bottleneck (GPSIMD cannot access PSUM, so the DVE
# takes both).  corr = -7 minimizes rms rel err (1.8%); +0.5 compensates the
# truncating float->int16 convert.  rel-err impact at 2/16 offload: ~+1e-3.
SCH_A = float(128.0 / np.log(2.0))
SCH_B = float(127 * 128 - 7 + 0.5)
SCH_KV = {5: "dve", 11: "dve"}

B, N, C, H = 4, 2048, 1024, 16
D = C // H          # 64
NCORES = 8
HL = H // 2         # 8 local heads per core
FL = HL * D         # 512 local features
KO = C // 128       # 8 contraction tiles (qkv proj)
KO2 = FL // 128     # 4 contraction tiles (out proj)
TB = N // 512       # 4 token blocks of 512
KV = N // 128       # 16 kv tiles of 128
QB = N // 512       # 4 query blocks of 512


# tuning knobs (timeline-sim sweeps)
PT_BUFS = 10
ST_BUFS = 2
PROJ_BUFS = 2
PV_PRIO_OFFSET = 60


# ---------------------------------------------------------------- launch
def _build_l1(reps=1):
    nc = bacc.Bacc("TRN2", target_bir_lowering=False, debug=False)
    xt = nc.dram_tensor("xt", [C, N], BF16, kind="ExternalInput")
    wq = nc.dram_tensor("wq", [C, FL], BF16, kind="ExternalInput")
    wk = nc.dram_tensor("wk", [C, FL], BF16, kind="ExternalInput")
    wv = nc.dram_tensor("wv", [C, FL], BF16, kind="ExternalInput")
    wp = nc.dram_tensor("wp", [FL, C], BF16, kind="ExternalInput")
    ident = nc.dram_tensor("ident", [128, 128], BF16, kind="ExternalInput")
    out = nc.dram_tensor("out", [N, C], F32, kind="ExternalOutput")

    xt_r = xt.ap().rearrange("(o p) n -> p o n", p=128)
    wq_r = wq.ap().rearrange("(o p) f -> p o f", p=128)
    wk_r = wk.ap().rearrange("(o p) f -> p o f", p=128)
    wv_r = wv.ap().rearrange("(o p) f -> p o f", p=128)
    wp_r = wp.ap().rearrange("(o p) c -> p o c", p=128)

    with (
        tile.TileContext(nc) as tc,
        tc.tile_pool(name="persist", bufs=1) as persist,
        tc.tile_pool(name="qk", bufs=2) as qk_p,
        tc.tile_pool(name="vp", bufs=2) as v_p,
        tc.tile_pool(name="pt", bufs=PT_BUFS) as pt_p,
        tc.tile_pool(name="onorm", bufs=4) as onorm_p,
        tc.tile_pool(name="rden", bufs=2) as rden_p,
        tc.tile_pool(name="outp", bufs=4) as outp,
        tc.tile_pool(name="ps_proj", bufs=PROJ_BUFS, space="PSUM") as ps_proj,
        tc.tile_pool(name="ps_st", bufs=ST_BUFS, space="PSUM") as ps_st,
        tc.tile_pool(name="ps_acc", bufs=1, space="PSUM") as ps_acc,
    ):
        xt_sb = persist.tile([128, KO, N], BF16)
        wq_sb = persist.tile([128, KO, FL], BF16)
        wk_sb = persist.tile([128, KO, FL], BF16)
        wv_sb = persist.tile([128, KO, FL], BF16)
        wp_sb = persist.tile([128, KO2, C], BF16)
        id_sb = persist.tile([128, 128], BF16)
        # oT[qb]: o_norm^T [512 local feats (4 ko-chunks of 128), 512 toks]
        oT = [persist.tile([128, KO2, 512], BF16, name=f"oT{qb}")
              for qb in range(QB)]
        # All loads on the SP queue in just-in-time order for the first
        # attention sweep (transfers serialize on the shared DMA engines, so
        # the queue order IS the arrival order): the head-pair-0 column
        # slices of the weights (cheap 128-col loads) + xt(tb0) enable the
        # first k/q/v chunks by ~6us, xt(tb1..3) arrive right before the
        # k(tb1..3) chunks need them, and the remaining weight columns
        # trail in (first needed by head-pair 1, ~70us later).  w_proj and
        # the transpose identity are first needed at the hp0-qb0 epilogue
        # (~80us in), so they ride at the back.
        nc.sync.dma_start(wk_sb[:, :, 0:128], wk_r[:, :, 0:128])
        nc.sync.dma_start(xt_sb[:, :, 0:256], xt_r[:, :, 0:256])
        nc.sync.dma_start(xt_sb[:, :, 256:512], xt_r[:, :, 256:512])
        nc.sync.dma_start(wq_sb[:, :, 0:128], wq_r[:, :, 0:128])
        nc.sync.dma_start(wv_sb[:, :, 0:128], wv_r[:, :, 0:128])
        for hb in range(2, 2 * TB):
            nc.sync.dma_start(xt_sb[:, :, hb * 256:(hb + 1) * 256],
                              xt_r[:, :, hb * 256:(hb + 1) * 256])
        nc.sync.dma_start(wk_sb[:, :, 128:], wk_r[:, :, 128:])
        nc.sync.dma_start(wq_sb[:, :, 128:], wq_r[:, :, 128:])
        nc.sync.dma_start(wv_sb[:, :, 128:], wv_r[:, :, 128:])
        nc.sync.dma_start(id_sb[:], ident.ap())
        nc.sync.dma_start(wp_sb[:], wp_r[:])

        for _rep in range(reps):
            def proj_chunks(hp, qT, kT, vA):
                """Generator: project k, v, q of head-pair hp in ~1us chunks.

                Chunk order is just-in-time for the first attention sweep
                (qb0 over kv 0..15): k(tb0) and q(tb0) first so scores can
                start immediately, then v in kv order interleaved with the
                remaining k blocks, then the remaining q blocks.
                """
                fsl = slice(hp * 128, (hp + 1) * 128)

                def kq(w_sb, dstT, tb, nm, half):
                    tok = slice(tb * 512 + half * 256,
                                tb * 512 + (half + 1) * 256)
                    p = ps_proj.tile([128, 256], F32, tag="proj",
                                     name=f"ps{nm}{hp}_{tb}_{half}")
                    for ko in range(KO):
                        nc.tensor.matmul(p[:], w_sb[:, ko, fsl],
                                         xt_sb[:, ko, tok],
                                         start=(ko == 0), stop=(ko == KO - 1))
                    nc.vector.tensor_copy(dstT[:, tok], p[:])

                def v_tile(tt):
                    tok = slice(tt * 128, (tt + 1) * 128)
                    psv = ps_proj.tile([128, 128], F32, tag="proj",
                                       name=f"psv{hp}_{tt}")
                    for ko in range(KO):
                        nc.tensor.matmul(psv[:], xt_sb[:, ko, tok],
                                         wv_sb[:, ko, fsl],
                                         start=(ko == 0), stop=(ko == KO - 1))
                    dst = vA[:, tt, :].rearrange("p (l c) -> p l c", l=2)
                    src = psv.rearrange("p (l c) -> p l c", l=2)
                    nc.vector.tensor_copy(dst[:, :, 0:64], src[:])

                for half in (0, 1):
                    kq(wk_sb, kT, 0, "k", half)
                    yield
                for half in (0, 1):
                    kq(wq_sb, qT, 0, "q", half)
                    yield
                nc.vector.memset(vA[:, :, 64], 1.0)
                nc.vector.memset(vA[:, :, 129], 1.0)
                for grp in range(TB):
                    if grp > 0:
                        for half in (0, 1):
                            kq(wk_sb, kT, grp, "k", half)
                            yield
                    for tt in range(grp * 4, grp * 4 + 4):
                        v_tile(tt)
                        yield
                for tb in range(1, TB):
                    for half in (0, 1):
                        kq(wq_sb, qT, tb, "q", half)
                        yield

            def attn_steps(hp, qT, kT, vA):
                """Generator: attention for head-pair hp, one kv step or one
                epilogue per yield.

                The PV accumulators hold 4 q-subtile chains per PSUM bank.
                A matmul with start=True lazily zeroes its bank's ENTIRE 2KB
                zero region; each acc tile owns its whole bank, and the PE
                executes its queue in order, so the FIRST PV matmul of the
                bank (kv0, sub0) zeroes it with start=True and every other
                PV matmul accumulates (start=False, group check off) --
                adds commute, so any execution order of the disjoint
                sub-chains is correct.  (A DVE memset would ride behind the
                previous q-block's epilogue in DVE program order and stall
                the first PV chains ~2us at every q-block boundary.)
                """
                for qb in range(QB):
                    qsl = slice(qb * 512, (qb + 1) * 512)
                    accs = [ps_acc.tile([128, 4, 65], F32, tag=f"acc{h}",
                                        name=f"acc{h}_{hp}_{qb}")
                            for h in (0, 1)]
                    for kv in range(KV):
                        ksl = slice(kv * 128, (kv + 1) * 128)
                        st = ps_st.tile([128, 2, 512], F32, tag="st",
                                        name=f"st{hp}_{qb}_{kv}")
                        for h in (0, 1):
                            hsl = slice(h * 64, (h + 1) * 64)
                            nc.tensor.matmul(st[:, h, :], kT[hsl, ksl],
                                             qT[hsl, qsl],
                                             start=True, stop=True)
                        off = SCH_KV.get(kv)
                        if off is None:
                            pt = pt_p.tile([128, 2, 512], BF16, tag="pt",
                                           name=f"pt{hp}_{qb}_{kv}")
                            nc.scalar.activation(
                                pt[:], st[:],
                                mybir.ActivationFunctionType.Exp)
                        else:
                            pti = pt_p.tile([128, 2, 512], I16, tag="pti",
                                            bufs=4, name=f"pti{hp}_{qb}_{kv}")
                            nc.vector.tensor_scalar(pti[:], st[:],
                                                    SCH_A, SCH_B,
                                                    mybir.AluOpType.mult,
                                                    mybir.AluOpType.add)
                            pt = pti.bitcast(BF16)
                        # Deprioritize PV: when both are ready the PE should
                        # run the ACT-feeding score/proj work first; the pt
                        # pool gives PV ~PT_BUFS steps of laxity and pt-slot
                        # pressure self-balances.
                        po = tc.cur_priority
                        tc.cur_priority = po + PV_PRIO_OFFSET
                        for h in (0, 1):
                            vsl = slice(h * 65, (h + 1) * 65)
                            for sub in range(4):
                                nc.tensor.matmul(
                                    accs[h][:, sub, :],
                                    pt[:, h, sub * 128:(sub + 1) * 128],
                                    vA[:, kv, vsl],
                                    start=(kv == 0 and sub == 0),
                                    stop=(kv == KV - 1),
                                    skip_group_check=True)
                        tc.cur_priority = po
                        yield
                    # Epilogue: normalize o = num/den on the PSUM->SBUF
                    # copy, then transpose to oT[qb][:, hp, :].
                    rden = rden_p.tile([128, 2, 4], F32, tag="rden",
                                       name=f"rden{hp}_{qb}")
                    onrm = onorm_p.tile([128, 4, 2, 64], BF16, tag="onorm",
                                        name=f"onrm{hp}_{qb}")
                    nc.vector.reciprocal(rden[:, 0, :], accs[0][:, :, 64])
                    for sub in range(4):
                        nc.vector.tensor_scalar_mul(
                            onrm[:, sub, 0, :], accs[0][:, sub, 0:64],
                            rden[:, 0, sub:sub + 1])
                    yield
                    nc.vector.reciprocal(rden[:, 1, :], accs[1][:, :, 64])
                    for sub in range(4):
                        nc.vector.tensor_scalar_mul(
                            onrm[:, sub, 1, :], accs[1][:, sub, 0:64],
                            rden[:, 1, sub:sub + 1])
                    # The transposes (PE) depend on the DVE normalize chain;
                    # emitting them here would park them at the head of the
                    # PE's in-order queue and stall the next q-block's
                    # scores.  Defer them to the pending work queue instead
                    # (pulled a step or two into the next sweep).
                    pending.append([-1, transpose_steps(hp, qb, onrm), 0])
                    yield

            def transpose_steps(hp, qb, onrm):
                tp = ps_proj.tile([128, 4, 128], BF16, tag="proj",
                                  name=f"tp{hp}_{qb}")
                for sub in range(4):
                    nc.tensor.matmul(tp[:, sub, :],
                                     onrm[:, sub, :, :], id_sb[:],
                                     is_transpose=True,
                                     start=(sub == 0), stop=(sub == 3),
                                     skip_group_check=True)
                nc.vector.tensor_copy(
                    oT[qb][:, hp, :],
                    tp.rearrange("p a b -> p (a b)"))
                yield

            def oproj_steps(qb):
                """Generator: partial out[qsl, :] = oT_qb^T @ w_proj_local,
                one [128 tok, 256 cout] PSUM tenure per yield."""
                for tt in range(4):
                    tsl = slice(tt * 128, (tt + 1) * 128)
                    osl = slice(qb * 512 + tt * 128, qb * 512 + (tt + 1) * 128)
                    for co in range(4):
                        csl = slice(co * 256, (co + 1) * 256)
                        ps = ps_proj.tile([128, 256], F32, tag="proj",
                                          name=f"op{qb}_{tt}_{co}")
                        for ko in range(KO2):
                            nc.tensor.matmul(ps[:], oT[qb][:, ko, tsl],
                                             wp_sb[:, ko, csl],
                                             start=(ko == 0),
                                             stop=(ko == KO2 - 1))
                        ob = outp.tile([128, 256], F32, tag="o",
                                       name=f"ob{qb}_{tt}_{co}")
                        nc.vector.tensor_copy(ob[:], ps[:])
                        nc.sync.dma_start(out.ap()[osl, csl], ob[:])
                        yield

            def hp_tiles(hp):
                qT = qk_p.tile([128, N], BF16, tag="qT", name=f"qT{hp}")
                kT = qk_p.tile([128, N], BF16, tag="kT", name=f"kT{hp}")
                vA = v_p.tile([128, KV, 130], BF16, tag="vA", name=f"vA{hp}")
                return qT, kT, vA

            # Interleave projection chunks (~1 per attention step) with the
            # attention steps; the tile scheduler resolves real deps, the
            # emission order sets priorities.  Attention(hp0) is emitted
            # right after k(tb0)+q(tb0) so the ACT pipeline starts ~10us in.
            # Emission order IS dependency order for the tile framework: an
            # attention step must be emitted AFTER the proj chunks it reads.
            # need_chunks[step] = how many chunks of the CURRENT head-pair's
            # generator must be emitted before attention step `step` (chunk
            # order: k0 q0 v0-3 k1 v4-7 k2 v8-11 k3 v12-15 q1 q2 q3).
            vpos = [4, 5, 6, 7, 10, 11, 12, 13, 16, 17, 18, 19, 22, 23, 24, 25]

            def need_chunks(step):
                qb, within = divmod(step, KV + 2)
                if qb == 0:
                    return vpos[min(within, KV - 1)] + 1
                return 26 + 2 * min(qb, 3)

            from collections import deque
            cur = hp_tiles(0)
            pending = deque()
            emitted = {}   # hp -> proj chunks emitted so far (32 = all)

            def add_proj(hp_, gen):
                emitted[hp_] = 0
                pending.append([hp_, gen, 0])

            add_proj(0, proj_chunks(0, *cur))

            def pull_one():
                while pending:
                    ent = pending[0]
                    if next(ent[1], StopIteration) is StopIteration:
                        pending.popleft()
                    else:
                        ent[2] += 1
                        if ent[0] >= 0:
                            emitted[ent[0]] = ent[2]
                        return
            for hp in range(NHP := HL // 2):
                if hp < NHP - 1:
                    nxt = hp_tiles(hp + 1)
                    add_proj(hp + 1, proj_chunks(hp + 1, *nxt))
                else:
                    nxt = None
                agen = attn_steps(hp, *cur)
                for step in range(QB * (KV + 2)):
                    # hard requirement: current head-pair's chunks this
                    # attention step reads must already be emitted (FIFO
                    # pulls drain whatever sits ahead of them in the queue)
                    while pending and emitted[hp] < need_chunks(step):
                        pull_one()
                    # cadence fill: one chunk every 3rd step regardless of
                    # owner -- the need-driven pulls above already force
                    # everything an attention step reads, so eager draining
                    # only crowds the PE and slips ACT.  During the last
                    # head-pair the queue holds oproj tenures and there are
                    # no more ACT-feeding proj chunks, so drain every step.
                    if pending and (step % 3 == 0 or hp == NHP - 1):
                        pull_one()
                    next(agen, None)
                    if hp == NHP - 1 and step % (KV + 2) == KV + 1:
                        qb_done = step // (KV + 2)
                        pending.append([-1, oproj_steps(qb_done), 0])
                cur = nxt
            while pending:
                if next(pending[0][1], StopIteration) is StopIteration:
                    pending.popleft()

    nc.compile()
    return nc


# ---------------------------------------------------------------- runner
class _SpmdRunner:
    """jit-once SPMD runner over n cores (modeled on bass2jax.run_bass_via_pjrt)."""

    def __init__(self, nc, n_cores):
        import jax
        from jax.experimental.shard_map import shard_map
        from jax.sharding import Mesh, PartitionSpec
        from concourse.bass2jax import (_bass_exec_p, install_neuronx_cc_hook,
                                        partition_id_tensor)

        install_neuronx_cc_hook()
        self.jax = jax
        self.n_cores = n_cores
        partition_name = (nc.partition_id_tensor.name
                          if nc.partition_id_tensor else None)
        in_names, out_names, out_avals, zero_shapes = [], [], [], []
        for alloc in nc.m.functions[0].allocations:
            if not isinstance(alloc, mybir.MemoryLocationSet):
                continue
            name = alloc.memorylocations[0].name
            if alloc.kind == "ExternalInput":
                if name != partition_name:
                    in_names.append(name)
            elif alloc.kind == "ExternalOutput":
                shape = tuple(alloc.tensor_shape)
                dtype = mybir.dt.np(alloc.dtype)
                out_names.append(name)
                out_avals.append(jax.core.ShapedArray(shape, dtype))
                zero_shapes.append((shape, dtype))
        self.in_names, self.out_names = in_names, out_names
        self.out_avals, self.zero_shapes = out_avals, zero_shapes
        n_params, n_outs = len(in_names), len(out_names)
        all_in = list(in_names) + list(out_names)
        if partition_name is not None:
            all_in.append(partition_name)

        def _body(*args):
            operands = list(args)
            if partition_name is not None:
                operands.append(partition_id_tensor())
            return tuple(_bass_exec_p.bind(
                *operands, out_avals=tuple(out_avals), in_names=tuple(all_in),
                out_names=tuple(out_names), lowering_input_output_aliases=(),
                sim_require_finite=True, sim_require_nnan=True, nc=nc))

        devices = jax.devices()[:n_cores]
        self.mesh = Mesh(np.asarray(devices), ("core",))
        self.pspec = PartitionSpec("core")
        in_specs = (self.pspec,) * (n_params + n_outs)
        out_specs = (self.pspec,) * n_outs
        self.fn = jax.jit(
            shard_map(_body, mesh=self.mesh, in_specs=in_specs,
                      out_specs=out_specs, check_rep=False),
            donate_argnums=tuple(range(n_params, n_params + n_outs)),
            keep_unused=True)

    def _stage(self, in_maps):
        from jax.sharding import NamedSharding
        sharding = NamedSharding(self.mesh, self.pspec)
        concat = [np.concatenate([np.asarray(m[n]) for m in in_maps], axis=0)
                  for n in self.in_names]
        dev_in = [self.jax.device_put(x, sharding) for x in concat]
        for x in dev_in:
            x.block_until_ready()
        return sharding, dev_in

    def _zeros(self, sharding):
        zeros = [self.jax.device_put(
            np.zeros((self.n_cores * s[0], *s[1:]), d), sharding)
            for (s, d) in self.zero_shapes]
        for z in zeros:
            z.block_until_ready()
        return zeros

    def _unpack(self, outs):
        np_outs = [np.asarray(o) for o in outs]
        return [
            {n: np_outs[i].reshape(self.n_cores, *self.out_avals[i].shape)[c]
             for i, n in enumerate(self.out_names)}
            for c in range(self.n_cores)
        ]

    def run(self, in_maps):
        sharding, dev_in = self._stage(in_maps)
        outs = self.fn(*dev_in, *self._zeros(sharding))
        return self._unpack(outs)

    def timed_run(self, in_maps, iters=6):
        """Stage inputs once; time only execute+sync per iteration."""
        import time
        sharding, dev_in = self._stage(in_maps)
        walls = []
        outs = None
        for _ in range(iters):
            zeros = self._zeros(sharding)
            t0 = time.perf_counter()
            outs = self.fn(*dev_in, *zeros)
            for o in outs:
                o.block_until_ready()
            walls.append(time.perf_counter() - t0)
        return self._unpack(outs), walls


_STATE = {}


def _get_state():
    if "l1" not in _STATE:
        nc1 = _build_l1()
        _STATE["l1"] = nc1
        _STATE["r1"] = _SpmdRunner(nc1, NCORES)
    return _STATE


def _l1_in_maps(x, w_qkv, w_proj):
    scale = np.float32(D ** -0.5)
    ident = np.eye(128, dtype=NP_BF16)
    in_maps = []
    for c in range(NCORES):
        b = c // 2
        hg = c % 2
        fsl = slice(hg * FL, (hg + 1) * FL)
        in_maps.append({
            "xt": np.ascontiguousarray(x[b].T).astype(NP_BF16),
            "wq": (np.ascontiguousarray(w_qkv[:, fsl]) * scale).astype(NP_BF16),
            "wk": np.ascontiguousarray(w_qkv[:, C:][:, fsl]).astype(NP_BF16),
            "wv": np.ascontiguousarray(w_qkv[:, 2 * C:][:, fsl]).astype(NP_BF16),
            "wp": np.ascontiguousarray(w_proj[fsl, :]).astype(NP_BF16),
            "ident": ident,
        })
    return in_maps


def kernel(x, w_qkv, w_proj, b_proj):
    st = _get_state()
    x = np.asarray(x, dtype=np.float32)
    w_qkv = np.asarray(w_qkv, dtype=np.float32)
    w_proj = np.asarray(w_proj, dtype=np.float32)
    b_proj = np.asarray(b_proj, dtype=np.float32)

    res = st["r1"].run(_l1_in_maps(x, w_qkv, w_proj))

    # host: sum the two head-group partials per batch, add bias
    out = np.empty((B, N, C), dtype=np.float32)
    for b in range(B):
        out[b] = res[2 * b]["out"] + res[2 * b + 1]["out"] + b_proj
    return out
